# revision 4
# baseline (speedup 1.0000x reference)
"""Trainium2 Bass kernel for nn_MultiHeadDistanceLayer.

Math: the reference's band-extract + avg-pool + position-sum reduces to
weighted diagonal sums of the per-(head,batch) attention matrix:

    S[h,b,d] = sum_{q} E[q, q+d] * r[q] * vr[q+d]        (d = 0..L-1)
    out[b,d,h] = (S[d-1] + S[d] + S[d+1]) / cnt[d]        (3-tap avg pool)

with E = exp(scores/8) (softmax max-subtraction is safe to skip for this
input scale), r = 1/rowsum(E), vr[k] = sigmoid(x @ Wv)[L-1-k].

Diagonal sums are computed by writing E*vr (bf16) to DRAM row-by-row with
row pitch Ls, then re-reading with a strided "skew" access pattern
(partition stride Ls+1) that turns diagonals into columns; a TensorEngine
matmul against the r-vector then reduces over the partition (q) axis
directly into PSUM.

Sharding: core c handles batch b = c//2 and heads 4*(c%2) .. 4*(c%2)+3.
Fully independent per core; host slices inputs and re-interleaves outputs.
"""

import os
import sys

for _p in ("/opt/trn_rl_repo", "/root/.axon_site/_ro/trn_rl_repo"):
    if os.path.isdir(_p) and _p not in sys.path:
        sys.path.insert(0, _p)

import numpy as np
from contextlib import ExitStack

import concourse.bass as bass
import concourse.bacc as bacc
import concourse.mybir as mybir
from concourse import tile
from concourse.bass_utils import run_bass_kernel_spmd

F32 = mybir.dt.float32
BF16 = mybir.dt.bfloat16
EXP = mybir.ActivationFunctionType.Exp

B, L, D, H, HD = 4, 1024, 256, 8, 64
NQ = L // 128          # 8 q-tiles of 128 rows
PAD = 130              # zero pad per scratch row (>= 127 + safety)
HPC = 4                # heads per core
NCORES = 8


def dmax(i):
    return L - 128 * i


def ls(i):
    return dmax(i) + PAD


def windows(i):
    """(w, d0, width) for q-tile i."""
    out = []
    d = dmax(i)
    if d > 0:
        out.append((0, 0, min(512, d)))
    if d > 512:
        out.append((1, 512, d - 512))
    return out


def build_nc():
    nc = bacc.Bacc("TRN2", target_bir_lowering=False, debug=False)

    xT = nc.dram_tensor("xT", [D, L], F32, kind="ExternalInput")
    xTr = nc.dram_tensor("xTr", [D, L], F32, kind="ExternalInput")
    peT = nc.dram_tensor("peT", [D, L], F32, kind="ExternalInput")
    wq = nc.dram_tensor("wq", [D, HPC * HD], F32, kind="ExternalInput")
    wk = nc.dram_tensor("wk", [D, HPC * HD], F32, kind="ExternalInput")
    wv = nc.dram_tensor("wv", [D, HPC], F32, kind="ExternalInput")
    bqT = nc.dram_tensor("bqT", [HD, HPC], F32, kind="ExternalInput")
    bkT = nc.dram_tensor("bkT", [HD, HPC], F32, kind="ExternalInput")
    out_part = nc.dram_tensor("out_part", [HPC, L], F32, kind="ExternalOutput")

    scratch = [[nc.dram_tensor(f"scr_{h}_{i}", [128 * ls(i)], BF16)
                for i in range(NQ)] for h in range(HPC)]

    with tile.TileContext(nc) as tc, ExitStack() as ctx:

        persist = ctx.enter_context(tc.tile_pool(name="persist", bufs=1))
        work = ctx.enter_context(tc.tile_pool(name="work", bufs=3))
        band_pool = ctx.enter_context(tc.tile_pool(name="band", bufs=4))
        psum_mm = ctx.enter_context(
            tc.tile_pool(name="psum_mm", bufs=2, space="PSUM"))
        psum_s = ctx.enter_context(
            tc.tile_pool(name="psum_s", bufs=4, space="PSUM"))

        # ---- load inputs ----
        xt = [persist.tile([128, L], F32, tag=f"xt{c}", name=f"xt{c}") for c in range(2)]
        pet = [persist.tile([128, L], F32, tag=f"pet{c}", name=f"pet{c}") for c in range(2)]
        xrt = [persist.tile([128, L], F32, tag=f"xrt{c}", name=f"xrt{c}") for c in range(2)]
        wqt = [persist.tile([128, HPC * HD], F32, tag=f"wqt{c}", name=f"wqt{c}") for c in range(2)]
        wkt = [persist.tile([128, HPC * HD], F32, tag=f"wkt{c}", name=f"wkt{c}") for c in range(2)]
        wvt = [persist.tile([128, HPC], F32, tag=f"wvt{c}", name=f"wvt{c}") for c in range(2)]
        for c in range(2):
            sl = slice(128 * c, 128 * (c + 1))
            nc.sync.dma_start(xt[c][:], xT[sl, :])
            nc.sync.dma_start(pet[c][:], peT[sl, :])
            nc.sync.dma_start(xrt[c][:], xTr[sl, :])
            nc.sync.dma_start(wqt[c][:], wq[sl, :])
            nc.sync.dma_start(wkt[c][:], wk[sl, :])
            nc.sync.dma_start(wvt[c][:], wv[sl, :])
        bqt = persist.tile([HD, HPC], F32, tag="bqt")
        bkt = persist.tile([HD, HPC], F32, tag="bkt")
        nc.sync.dma_start(bqt[:], bqT[:])
        nc.sync.dma_start(bkt[:], bkT[:])

        # ---- pre-zero scratch pad gaps ----
        zpad = persist.tile([128, PAD], BF16, tag="zpad")
        nc.vector.memset(zpad[:], 0.0)
        for h in range(HPC):
            for i in range(NQ):
                pad_ap = bass.AP(scratch[h][i], dmax(i), [[ls(i), 128], [1, PAD]])
                nc.sync.dma_start(pad_ap, zpad[:])

        # ---- xpe = x + pe (bf16) ----
        xpe = [persist.tile([128, L], BF16, tag=f"xpe{c}", name=f"xpe{c}") for c in range(2)]
        for c in range(2):
            nc.vector.tensor_add(xpe[c][:], xt[c][:], pet[c][:])

        # ---- bf16 weight casts ----
        wqb = [persist.tile([128, HPC * HD], BF16, tag=f"wqb{c}", name=f"wqb{c}") for c in range(2)]
        wkb = [persist.tile([128, HPC * HD], BF16, tag=f"wkb{c}", name=f"wkb{c}") for c in range(2)]
        for c in range(2):
            nc.vector.tensor_copy(wqb[c][:], wqt[c][:])
            nc.vector.tensor_copy(wkb[c][:], wkt[c][:])

        # ---- v = sigmoid(x_rev @ Wv) = 1/(1+exp(-x_rev @ Wv)), f32 ----
        vpsum = psum_mm.tile([HPC, L], F32, tag="mm")
        for half in range(2):
            hs = slice(512 * half, 512 * (half + 1))
            for c in range(2):
                nc.tensor.matmul(vpsum[:, hs], wvt[c][:], xrt[c][:, hs],
                                 start=(c == 0), stop=(c == 1))
        vexp = work.tile([HPC, L], F32, tag="vtmp")
        nc.scalar.activation(vexp[:], vpsum[:], EXP, scale=-1.0)
        nc.vector.tensor_scalar_add(vexp[:], vexp[:], 1.0)
        vrec = work.tile([HPC, L], F32, tag="vrec")
        nc.vector.reciprocal(vrec[:], vexp[:])
        vrb = persist.tile([HPC, L], BF16, tag="vrb")
        nc.vector.tensor_copy(vrb[:], vrec[:])
        # move each head row to partition 0 for partition_broadcast
        vrow = [persist.tile([1, L], BF16, tag=f"vrow{h}", name=f"vrow{h}") for h in range(HPC)]
        for h in range(HPC):
            nc.sync.dma_start(vrow[h][:], vrb[h:h + 1, :])

        # ---- Q/K projections: QT[h] = (Wq_h^T @ xpe^T) + bq_h, bf16 [64, L] ----
        qt = [persist.tile([HD, L], BF16, tag=f"qt{h}", name=f"qt{h}") for h in range(HPC)]
        kt = [persist.tile([HD, L], BF16, tag=f"kt{h}", name=f"kt{h}") for h in range(HPC)]
        for h in range(HPC):
            hs = slice(HD * h, HD * (h + 1))
            for dst, wb, bt in ((qt, wqb, bqt), (kt, wkb, bkt)):
                ppsum = psum_mm.tile([HD, L], F32, tag="mm")
                for half in range(2):
                    fs = slice(512 * half, 512 * (half + 1))
                    for c in range(2):
                        nc.tensor.matmul(ppsum[:, fs], wb[c][:, hs],
                                         xpe[c][:, fs],
                                         start=(c == 0), stop=(c == 1))
                nc.vector.tensor_scalar_add(dst[h][:], ppsum[:], bt[:, h:h + 1])

        # ---- per-(head) main pipeline ----
        srows = persist.tile([HPC, L], F32, tag="srows")
        for h in range(HPC):
            vbc = work.tile([128, L], BF16, tag="vbc")
            nc.gpsimd.partition_broadcast(vbc[:], vrow[h][:])
            spsum = [psum_s.tile([1, 512], F32, tag="spsum", name="spsum") for _ in range(2)]
            for i in range(NQ):
                q0 = 128 * i
                sc = psum_mm.tile([128, L], F32, tag="mm")
                for half in range(2):
                    fs = slice(512 * half, 512 * (half + 1))
                    nc.tensor.matmul(sc[:, fs], qt[h][:, q0:q0 + 128],
                                     kt[h][:, fs], start=True, stop=True)
                et = work.tile([128, L], BF16, tag="et")
                zacc = work.tile([128, 1], F32, tag="zacc")
                nc.scalar.activation(et[:], sc[:], EXP, scale=0.125,
                                     accum_out=zacc[:])
                rf = work.tile([128, 1], F32, tag="rf")
                nc.vector.reciprocal(rf[:], zacc[:])
                rb = work.tile([128, 1], BF16, tag="rb")
                nc.vector.tensor_copy(rb[:], rf[:])
                ev = work.tile([128, dmax(i)], BF16, tag="ev")
                nc.vector.tensor_tensor(ev[:], et[:, q0:], vbc[:, q0:],
                                        op=mybir.AluOpType.mult)
                wr_ap = bass.AP(scratch[h][i], 0, [[ls(i), 128], [1, dmax(i)]])
                nc.sync.dma_start(wr_ap, ev[:])
                for (w, d0, width) in windows(i):
                    bt = band_pool.tile([128, width], BF16, tag="bt")
                    rd_ap = bass.AP(scratch[h][i], d0,
                                    [[ls(i) + 1, 128], [1, width]])
                    nc.sync.dma_start(bt[:], rd_ap)
                    last_i = NQ - 1 if w == 0 else 3
                    nc.tensor.matmul(spsum[w][:, :width], rb[:], bt[:],
                                     start=(i == 0), stop=(i == last_i))
            for w in range(2):
                scopy = work.tile([1, 512], F32, tag="scopy", name="scopy")
                nc.vector.tensor_copy(scopy[:], spsum[w][:])
                nc.sync.dma_start(srows[h:h + 1, 512 * w:512 * (w + 1)],
                                  scopy[:])

        # ---- 3-tap average pool (TF SAME, count excludes padding) ----
        invc = persist.tile([HPC, L], F32, tag="invc")
        nc.vector.memset(invc[:], 1.0 / 3.0)
        nc.vector.memset(invc[:, 0:1], 0.5)
        nc.vector.memset(invc[:, L - 1:L], 0.5)
        t1 = persist.tile([HPC, L], F32, tag="t1")
        nc.vector.tensor_add(t1[:, 1:], srows[:, :L - 1], srows[:, 1:])
        nc.vector.tensor_copy(t1[:, 0:1], srows[:, 0:1])
        t2 = persist.tile([HPC, L], F32, tag="t2")
        nc.vector.tensor_add(t2[:, :L - 1], t1[:, :L - 1], srows[:, 1:])
        nc.vector.tensor_copy(t2[:, L - 1:L], t1[:, L - 1:L])
        t3 = persist.tile([HPC, L], F32, tag="t3")
        nc.vector.tensor_tensor(t3[:], t2[:], invc[:],
                                op=mybir.AluOpType.mult)
        nc.sync.dma_start(out_part[:], t3[:])

    nc.compile()
    return nc


_NC_CACHE = None


def _get_nc():
    global _NC_CACHE
    if _NC_CACHE is None:
        _NC_CACHE = build_nc()
    return _NC_CACHE


def make_in_maps(x, Wq, bq, Wk, bk, Wv, pe):
    x = np.asarray(x, np.float32)
    Wq = np.asarray(Wq, np.float32)
    Wk = np.asarray(Wk, np.float32)
    Wv = np.asarray(Wv, np.float32)
    pe = np.asarray(pe, np.float32)
    bq = np.asarray(bq, np.float32).reshape(H, HD)
    bk = np.asarray(bk, np.float32).reshape(H, HD)
    peT = np.ascontiguousarray(pe.T)
    in_maps = []
    for c in range(NCORES):
        b, hg = c // 2, c % 2
        hs = slice(HPC * HD * hg, HPC * HD * (hg + 1))
        hh = slice(HPC * hg, HPC * (hg + 1))
        xb = x[b]
        in_maps.append({
            "xT": np.ascontiguousarray(xb.T),
            "xTr": np.ascontiguousarray(xb[::-1].T),
            "peT": peT,
            "wq": np.ascontiguousarray(Wq[:, hs]),
            "wk": np.ascontiguousarray(Wk[:, hs]),
            "wv": np.ascontiguousarray(Wv[:, hh]),
            "bqT": np.ascontiguousarray(bq[hh].T),
            "bkT": np.ascontiguousarray(bk[hh].T),
        })
    return in_maps


def assemble(results):
    out = np.zeros((B, L, H), np.float32)
    for c in range(NCORES):
        b, hg = c // 2, c % 2
        part = np.asarray(results[c]["out_part"], np.float32)
        for j in range(HPC):
            out[b, :, HPC * hg + j] = part[j]
    return out


def kernel(x, Wq, bq, Wk, bk, Wv, pe):
    nc = _get_nc()
    in_maps = make_in_maps(x, Wq, bq, Wk, bk, Wv, pe)
    res = run_bass_kernel_spmd(nc, in_maps, list(range(NCORES)))
    return assemble(res.results)
